# revision 20
# baseline (speedup 1.0000x reference)
"""Causal self-attention with RoPE, tensor-parallel over (batch, head-group)
across 8 NeuronCores.

Sharding: core c = 4*b + g handles batch b (of 2) and head group g (of 4),
i.e. heads 4g..4g+3.  Each core computes q/k projections in transposed
layout [head_dim, seq] (weights become matmul lhsT naturally), v in natural
layout [seq, head_dim], applies RoPE, runs causal attention without
max-subtraction (scores are O(3), exp is safe in fp32), and emits a partial
output projection.  The host sums the 4 per-head-group partials per batch.

All matmul operands are fp16 (full PE rate, f32 PSUM accumulation); the
non-matmul math (RoPE, exp, reciprocal) stays f32.
"""

import sys
from contextlib import ExitStack

sys.path.insert(0, "/opt/trn_rl_repo")

import numpy as np

import concourse.bass as bass
import concourse.tile as tile
from concourse import bacc, mybir

B, S, D, H, HD = 2, 2048, 2048, 16, 128
NCORES = 8
HPC = H // 4  # heads per core = 4
DG = HPC * HD  # 512 cols per head group
P = 128
SB = 512  # s-block (matmul free dim)
NSB = S // SB  # 4
NDT = D // P  # 16 contraction tiles of the model dim
NST = S // P  # 16 seq tiles
F32 = mybir.dt.float32
F32R = mybir.dt.float32r
MMDT = mybir.dt.float16
MMNP = np.float16
SCALE = 1.0 / float(np.sqrt(HD))


def _build_program(with_qkv_bias: bool):
    nc = bacc.Bacc("TRN2", target_bir_lowering=False, debug=False,
                   num_devices=NCORES)
    xT = nc.dram_tensor("xT", [D, S], MMDT, kind="ExternalInput").ap()
    wq = nc.dram_tensor("wq", [D, DG], MMDT, kind="ExternalInput").ap()
    wk = nc.dram_tensor("wk", [D, DG], MMDT, kind="ExternalInput").ap()
    wv = nc.dram_tensor("wv", [D, DG], MMDT, kind="ExternalInput").ap()
    wo = nc.dram_tensor("wo", [DG, D], MMDT, kind="ExternalInput").ap()
    cosT = nc.dram_tensor("cosT", [P, S], F32, kind="ExternalInput").ap()
    sinST = nc.dram_tensor("sinST", [P, S], F32, kind="ExternalInput").ap()
    masksT = nc.dram_tensor("masksT", [P, 2, 2, SB], MMDT,
                            kind="ExternalInput").ap()
    onesd = nc.dram_tensor("ones", [P, P], MMDT, kind="ExternalInput").ap()
    if with_qkv_bias:
        bqr = nc.dram_tensor("bqrope", [P, HPC, S], F32, kind="ExternalInput").ap()
        bkr = nc.dram_tensor("bkrope", [P, HPC, S], F32, kind="ExternalInput").ap()
        bv128 = nc.dram_tensor("bv128", [P, DG], F32, kind="ExternalInput").ap()
    out = nc.dram_tensor("out", [S, D], F32, kind="ExternalOutput").ap()

    with tile.TileContext(nc) as tc:
        with ExitStack() as top:
            # ---- persistent tiles ----
            qkT_pool = top.enter_context(tc.tile_pool(name="qkT", bufs=1))
            qT = qkT_pool.tile([P, HPC, S], MMDT, tag="qT")
            kT = qkT_pool.tile([P, HPC, S], MMDT, tag="kT")
            v_pool = top.enter_context(tc.tile_pool(name="vp", bufs=1))
            vN = v_pool.tile([P, NST, DG], MMDT, tag="vN")

            # ---- phase A: q/k (transposed) + v (natural) projections ----
            # Three 4-bank PE waves per s-block (q, k, v); with 8 PSUM banks
            # two waves are in flight so RoPE/copy eviction of wave i
            # overlaps wave i+1's matmuls and the PE never idles.
            with ExitStack() as actx:
                wpool = actx.enter_context(tc.tile_pool(name="wqkv", bufs=1))
                wq_t = wpool.tile([P, NDT, DG], MMDT, tag="wq")
                wk_t = wpool.tile([P, NDT, DG], MMDT, tag="wk")
                wv_t = wpool.tile([P, NDT, DG], MMDT, tag="wv")
                # per-dt slices so the first matmuls start without waiting
                # for the whole weight transfer
                for dt in range(NDT):
                    nc.sync.dma_start(wq_t[:, dt, :], wq[bass.ts(dt, P), :])
                    nc.sync.dma_start(wk_t[:, dt, :], wk[bass.ts(dt, P), :])
                    nc.sync.dma_start(wv_t[:, dt, :], wv[bass.ts(dt, P), :])
                cpool = actx.enter_context(tc.tile_pool(name="cs", bufs=1))
                cos_t = cpool.tile([P, S], F32, tag="cos")
                sin_t = cpool.tile([P, S], F32, tag="sin")
                nc.sync.dma_start(cos_t[:], cosT[:])
                nc.sync.dma_start(sin_t[:], sinST[:])
                if with_qkv_bias:
                    bpool = actx.enter_context(tc.tile_pool(name="bqk", bufs=1))
                    bqr_t = bpool.tile([P, HPC, S], F32, tag="bqr")
                    bkr_t = bpool.tile([P, HPC, S], F32, tag="bkr")
                    bv_t = bpool.tile([P, DG], F32, tag="bv")
                    nc.sync.dma_start(bqr_t[:], bqr[:])
                    nc.sync.dma_start(bkr_t[:], bkr[:])
                    nc.sync.dma_start(bv_t[:], bv128[:])
                xpool = actx.enter_context(tc.tile_pool(name="xs", bufs=32))
                tpool = actx.enter_context(tc.tile_pool(name="ropetmp", bufs=4))
                pspool = actx.enter_context(
                    tc.tile_pool(name="psA", bufs=8, space="PSUM"))

                for sb in range(NSB):
                    ssl = bass.ts(sb, SB)
                    xts = []
                    for dt in range(NDT):
                        xt = xpool.tile([P, SB], MMDT, tag="xs",
                                        name=f"x_{sb}_{dt}")
                        nc.sync.dma_start(xt[:], xT[bass.ts(dt, P), ssl])
                        xts.append(xt)

                    # wave q / wave k: transposed projection + RoPE
                    for wname, w_t, dst in (("q", wq_t, qT), ("k", wk_t, kT)):
                        ps = [pspool.tile([P, SB], F32, tag="psA",
                                          name=f"ps{wname}_{sb}_{h}")
                              for h in range(HPC)]
                        for dt in range(NDT):
                            for h in range(HPC):
                                nc.tensor.matmul(
                                    ps[h][:], w_t[:, dt, bass.ts(h, P)],
                                    xts[dt][:],
                                    start=(dt == 0), stop=(dt == NDT - 1))
                        for h in range(HPC):
                            p = ps[h]
                            tmp = tpool.tile([P, SB], F32, tag="ropetmp")
                            nc.vector.tensor_mul(
                                tmp[0:64, :], p[64:128, :], sin_t[0:64, ssl])
                            nc.vector.tensor_mul(
                                tmp[64:128, :], p[0:64, :], sin_t[64:128, ssl])
                            dst_ap = dst[:, h, ssl]
                            nc.vector.tensor_mul(dst_ap, p[:], cos_t[:, ssl])
                            nc.vector.tensor_add(dst_ap, dst_ap, tmp[:])
                            if with_qkv_bias:
                                bt = bqr_t if wname == "q" else bkr_t
                                nc.vector.tensor_add(dst_ap, dst_ap,
                                                     bt[:, h, ssl])

                    # wave v: natural projection, lhsT is a slice of xt
                    pv = [pspool.tile([P, DG], F32, tag="psA",
                                      name=f"psv_{sb}_{j}")
                          for j in range(4)]
                    for dt in range(NDT):
                        for j in range(4):
                            nc.tensor.matmul(
                                pv[j][:], xts[dt][:, bass.ts(j, P)],
                                wv_t[:, dt, :],
                                start=(dt == 0), stop=(dt == NDT - 1))
                    for j in range(4):
                        st = 4 * sb + j
                        if with_qkv_bias:
                            nc.vector.tensor_add(vN[:, st, :], pv[j][:],
                                                 bv_t[:])
                        else:
                            nc.vector.tensor_copy(vN[:, st, :], pv[j][:])

            # ---- phase C: causal attention per (head, i-block) ----
            # j-tiles processed in pairs; exp runs as one 1024-wide ACT op.
            oT_pool = top.enter_context(tc.tile_pool(name="oTp", bufs=1))
            oT = oT_pool.tile([P, HPC, S], MMDT, tag="oT")
            # prefetch the out-proj weights during attention
            wopool = top.enter_context(tc.tile_pool(name="wo", bufs=1))
            wo_t = wopool.tile([P, HPC, D], MMDT, tag="wo")
            for hh in range(HPC):
                nc.sync.dma_start(wo_t[:, hh, :], wo[bass.ts(hh, P), :])
            with ExitStack() as cctx:
                mpool = cctx.enter_context(tc.tile_pool(name="masks", bufs=1))
                mask_t = mpool.tile([P, 2, 2, SB], MMDT, tag="masks")
                nc.sync.dma_start(mask_t[:], masksT[:])
                ones_t = mpool.tile([P, P], MMDT, tag="ones")
                nc.sync.dma_start(ones_t[:], onesd[:])
                epool = cctx.enter_context(tc.tile_pool(name="et", bufs=4))
                rpool = cctx.enter_context(tc.tile_pool(name="recip", bufs=2))
                psS = cctx.enter_context(
                    tc.tile_pool(name="psS", bufs=2, space="PSUM"))
                psO = cctx.enter_context(
                    tc.tile_pool(name="psO", bufs=2, space="PSUM"))
                psD = cctx.enter_context(
                    tc.tile_pool(name="psD", bufs=2, space="PSUM"))

                for h in range(HPC):
                    for ib in range(NSB):
                        isl = bass.ts(ib, SB)
                        npair = 2 * (ib + 1)
                        po = psO.tile([P, SB], F32, tag="psO")
                        pd = psD.tile([P, SB], F32, tag="psD")
                        for pt in range(npair):
                            pss = psS.tile([P, 2, SB], F32, tag="psS")
                            for t in range(2):
                                nc.tensor.matmul(
                                    pss[:, t, :],
                                    kT[:, h, bass.ts(2 * pt + t, P)],
                                    qT[:, h, isl], start=True, stop=True)
                            et = epool.tile([P, 2, SB], MMDT, tag="et")
                            nc.scalar.activation(
                                et[:], pss[:],
                                mybir.ActivationFunctionType.Exp, scale=SCALE)
                            o = pt - 2 * ib
                            if o >= 0:
                                nc.vector.tensor_mul(et[:], et[:],
                                                     mask_t[:, o, :, :])
                            first = (pt == 0)
                            last = (pt == npair - 1)
                            for t in range(2):
                                nc.tensor.matmul(
                                    pd[:], ones_t[:], et[:, t, :],
                                    start=(first and t == 0),
                                    stop=(last and t == 1))
                                nc.tensor.matmul(
                                    po[:], vN[:, 2 * pt + t, bass.ts(h, P)],
                                    et[:, t, :],
                                    start=(first and t == 0),
                                    stop=(last and t == 1))
                        # 1/den on one row only, then matmul-broadcast (K=1)
                        # to all 128 partitions: lhsT=ones[0:1,:], rhs=rec1
                        rec1 = rpool.tile([1, SB], MMDT, tag="recip")
                        with nc.allow_low_precision(
                                reason="1/den fits fp16; den in [1, 2e3]"):
                            nc.vector.reciprocal(rec1[:], pd[0:1, :])
                        recb = psD.tile([P, SB], F32, tag="psD",
                                        name=f"recb_{h}_{ib}")
                        nc.tensor.matmul(recb[:], ones_t[0:1, :], rec1[:],
                                         start=True, stop=True)
                        recs = rpool.tile([P, SB], F32, tag="recs")
                        nc.vector.tensor_copy(recs[:], recb[:])
                        nc.vector.tensor_mul(oT[:, h, isl], po[:], recs[:])

            # ---- phase D: partial output projection ----
            with ExitStack() as dctx:
                opool = dctx.enter_context(tc.tile_pool(name="outsb", bufs=4))
                psE = dctx.enter_context(
                    tc.tile_pool(name="psE", bufs=2, space="PSUM"))
                for st in range(NST):
                    for eb in range(NSB):
                        pe = psE.tile([P, SB], F32, tag="psE")
                        for hh in range(HPC):
                            nc.tensor.matmul(
                                pe[:], oT[:, hh, bass.ts(st, P)],
                                wo_t[:, hh, bass.ts(eb, SB)],
                                start=(hh == 0), stop=(hh == HPC - 1))
                        ob = opool.tile([P, SB], F32, tag="outsb")
                        nc.vector.tensor_copy(ob[:], pe[:])
                        nc.sync.dma_start(
                            out[bass.ts(st, P), bass.ts(eb, SB)], ob[:])

    nc.compile()
    return nc


def _rot_cols(w):
    """rotate_half applied to the last axis (head-dim columns) of w."""
    r = np.empty_like(w)
    r[..., : HD // 2] = -w[..., HD // 2:]
    r[..., HD // 2:] = w[..., : HD // 2]
    return r


def _host_inputs(x, cos, sin, qkv_w, qkv_b, with_qkv_bias):
    """Build the 8 per-core input maps."""
    # signed sin, transposed: sinS[d] = -sin[d] for d<64 else +sin[d]
    sinS = sin.copy()
    sinS[:, : HD // 2] *= -1.0
    cosT = np.ascontiguousarray(cos.T)
    sinST = np.ascontiguousarray(sinS.T)
    masks = np.zeros((P, 2, 2, SB), dtype=MMNP)
    jj = np.arange(P)[:, None]
    ii = np.arange(SB)[None, :]
    for o in (0, 1):
        for t in (0, 1):
            masks[:, o, t, :] = (((2 * o + t) * P + jj) <= ii).astype(MMNP)
    ones = np.ones((P, P), dtype=MMNP)

    xTb = [np.ascontiguousarray(x[b].T).astype(MMNP) for b in range(B)]
    qkv_w16 = qkv_w.astype(MMNP)
    in_maps = []
    for c in range(NCORES):
        b, g = divmod(c, 4)
        cols = slice(g * DG, (g + 1) * DG)
        im = {
            "xT": xTb[b],
            "wq": np.ascontiguousarray(qkv_w16[:, cols]),
            "wk": np.ascontiguousarray(qkv_w16[:, D:][:, cols]),
            "wv": np.ascontiguousarray(qkv_w16[:, 2 * D:][:, cols]),
            "wo": None,  # filled by caller (needs out_w)
            "cosT": cosT,
            "sinST": sinST,
            "masksT": masks,
            "ones": ones,
        }
        if with_qkv_bias:
            bq = qkv_b[cols]
            bk = qkv_b[D:][cols]
            bv = qkv_b[2 * D:][cols]
            # roped bias, transposed per head: [HD, HPC, S]
            def rope_bias(bvec):
                r = np.empty((P, HPC, S), dtype=np.float32)
                for h in range(HPC):
                    bh = bvec[h * HD:(h + 1) * HD]  # [HD]
                    rb = _rot_cols(bh[None, :])[0]
                    # b*cos + rot(b)*sin, as [HD, S]
                    r[:, h, :] = (bh[None, :] * cos + rb[None, :] * sin).T
                return r
            im["bqrope"] = rope_bias(bq)
            im["bkrope"] = rope_bias(bk)
            im["bv128"] = np.tile(bv[None, :], (P, 1)).astype(np.float32)
        in_maps.append(im)
    return in_maps


_CACHED = {}


def _get_program(with_qkv_bias):
    if with_qkv_bias not in _CACHED:
        _CACHED[with_qkv_bias] = _build_program(with_qkv_bias)
    return _CACHED[with_qkv_bias]


def run_on_cores(in_maps, profile_dir=None):
    """Execute the prebuilt program on 8 cores; optionally capture NTFF."""
    from concourse import bass2jax
    with_qkv_bias = "bqrope" in in_maps[0]
    nc = _get_program(with_qkv_bias)
    if profile_dir is not None:
        from trn_agent_boot.trn_boot import _ntff_profile_via_ctypes
        hook = _ntff_profile_via_ctypes("/opt/axon/libaxon_pjrt.so")
        with hook(profile_dir, [0]):
            results = bass2jax.run_bass_via_pjrt(nc, in_maps, n_cores=NCORES)
    else:
        results = bass2jax.run_bass_via_pjrt(nc, in_maps, n_cores=NCORES)
    return results


def kernel(x, cos, sin, qkv_w, qkv_b, out_w, out_b, _profile_dir=None):
    x = np.asarray(x, dtype=np.float32)
    cos = np.asarray(cos, dtype=np.float32)
    sin = np.asarray(sin, dtype=np.float32)
    qkv_w = np.asarray(qkv_w, dtype=np.float32)
    qkv_b = np.asarray(qkv_b, dtype=np.float32)
    out_w = np.asarray(out_w, dtype=np.float32)
    out_b = np.asarray(out_b, dtype=np.float32)

    with_qkv_bias = bool(np.any(qkv_b != 0))
    in_maps = _host_inputs(x, cos, sin, qkv_w, qkv_b, with_qkv_bias)
    for c in range(NCORES):
        g = c % 4
        in_maps[c]["wo"] = np.ascontiguousarray(
            out_w[g * DG:(g + 1) * DG, :]).astype(MMNP)

    results = run_on_cores(in_maps, profile_dir=_profile_dir)

    final = np.zeros((B, S, D), dtype=np.float32)
    for c in range(NCORES):
        b = c // 4
        final[b] += results[c]["out"]
    final += out_b[None, None, :]
    return final


# revision 23
# speedup vs baseline: 1.1044x; 1.1044x over previous
"""Causal self-attention with RoPE, tensor-parallel over (batch, head-group)
across 8 NeuronCores.

Sharding: core c = 4*b + g handles batch b (of 2) and head group g (of 4),
i.e. heads 4g..4g+3.  Each core computes q/k projections in transposed
layout [head_dim, seq] (weights become matmul lhsT naturally), v in natural
layout [seq, head_dim], applies RoPE, runs causal attention without
max-subtraction (scores are O(3), exp is safe in fp32), and emits a partial
output projection.  The host sums the 4 per-head-group partials per batch.

All matmul operands are fp16 (full PE rate, f32 PSUM accumulation); the
non-matmul math (RoPE, exp, reciprocal) stays f32.
"""

import sys
from contextlib import ExitStack

sys.path.insert(0, "/opt/trn_rl_repo")

import numpy as np

import concourse.bass as bass
import concourse.tile as tile
from concourse import bacc, mybir

B, S, D, H, HD = 2, 2048, 2048, 16, 128
NCORES = 8
HPC = H // 4  # heads per core = 4
DG = HPC * HD  # 512 cols per head group
P = 128
SB = 512  # s-block (matmul free dim)
NSB = S // SB  # 4
NDT = D // P  # 16 contraction tiles of the model dim
NST = S // P  # 16 seq tiles
F32 = mybir.dt.float32
F32R = mybir.dt.float32r
MMDT = mybir.dt.float16
MMNP = np.float16
SCALE = 1.0 / float(np.sqrt(HD))


def _build_program(with_qkv_bias: bool):
    nc = bacc.Bacc("TRN2", target_bir_lowering=False, debug=False,
                   num_devices=NCORES)
    xT = nc.dram_tensor("xT", [D, S], MMDT, kind="ExternalInput").ap()
    wq = nc.dram_tensor("wq", [D, DG], MMDT, kind="ExternalInput").ap()
    wk = nc.dram_tensor("wk", [D, DG], MMDT, kind="ExternalInput").ap()
    wv = nc.dram_tensor("wv", [D, DG], MMDT, kind="ExternalInput").ap()
    wo = nc.dram_tensor("wo", [DG, D], MMDT, kind="ExternalInput").ap()
    cosT = nc.dram_tensor("cosT", [P, S], F32, kind="ExternalInput").ap()
    sinST = nc.dram_tensor("sinST", [P, S], F32, kind="ExternalInput").ap()
    masksT = nc.dram_tensor("masksT", [P, 2, 2, SB], MMDT,
                            kind="ExternalInput").ap()
    onesd = nc.dram_tensor("ones", [P, P], MMDT, kind="ExternalInput").ap()
    if with_qkv_bias:
        bqr = nc.dram_tensor("bqrope", [P, HPC, S], F32, kind="ExternalInput").ap()
        bkr = nc.dram_tensor("bkrope", [P, HPC, S], F32, kind="ExternalInput").ap()
        bv128 = nc.dram_tensor("bv128", [P, DG], F32, kind="ExternalInput").ap()
    out = nc.dram_tensor("out", [S, D], F32, kind="ExternalOutput").ap()

    with tile.TileContext(nc) as tc:
        with ExitStack() as top:
            # ---- persistent tiles ----
            qkT_pool = top.enter_context(tc.tile_pool(name="qkT", bufs=1))
            qT = qkT_pool.tile([P, HPC, S], MMDT, tag="qT")
            kT = qkT_pool.tile([P, HPC, S], MMDT, tag="kT")
            v_pool = top.enter_context(tc.tile_pool(name="vp", bufs=1))
            vN = v_pool.tile([P, NST, DG], MMDT, tag="vN")

            # ---- phase A: q/k (transposed) + v (natural) projections ----
            # Three 4-bank PE waves per s-block (q, k, v); with 8 PSUM banks
            # two waves are in flight so RoPE/copy eviction of wave i
            # overlaps wave i+1's matmuls and the PE never idles.
            with ExitStack() as actx:
                wpool = actx.enter_context(tc.tile_pool(name="wqkv", bufs=1))
                wq_t = wpool.tile([P, NDT, DG], MMDT, tag="wq")
                wk_t = wpool.tile([P, NDT, DG], MMDT, tag="wk")
                wv_t = wpool.tile([P, NDT, DG], MMDT, tag="wv")
                # wq first; wk/wv/cos/sin issued after sb0's x tiles so the
                # first q matmuls aren't queued behind 7MB of other DMA
                nc.sync.dma_start(wq_t[:], wq.rearrange("(t p) n -> p t n", p=P))
                cpool = actx.enter_context(tc.tile_pool(name="cs", bufs=1))
                cos_t = cpool.tile([P, S], F32, tag="cos")
                sin_t = cpool.tile([P, S], F32, tag="sin")
                if with_qkv_bias:
                    bpool = actx.enter_context(tc.tile_pool(name="bqk", bufs=1))
                    bqr_t = bpool.tile([P, HPC, S], F32, tag="bqr")
                    bkr_t = bpool.tile([P, HPC, S], F32, tag="bkr")
                    bv_t = bpool.tile([P, DG], F32, tag="bv")
                    nc.sync.dma_start(bqr_t[:], bqr[:])
                    nc.sync.dma_start(bkr_t[:], bkr[:])
                    nc.sync.dma_start(bv_t[:], bv128[:])
                xpool = actx.enter_context(tc.tile_pool(name="xs", bufs=32))
                tpool = actx.enter_context(tc.tile_pool(name="ropetmp", bufs=4))
                pspool = actx.enter_context(
                    tc.tile_pool(name="psA", bufs=8, space="PSUM"))

                for sb in range(NSB):
                    ssl = bass.ts(sb, SB)
                    xts = []
                    for dt in range(NDT):
                        xt = xpool.tile([P, SB], MMDT, tag="xs",
                                        name=f"x_{sb}_{dt}")
                        nc.sync.dma_start(xt[:], xT[bass.ts(dt, P), ssl])
                        xts.append(xt)
                    if sb == 0:
                        nc.sync.dma_start(
                            wk_t[:], wk.rearrange("(t p) n -> p t n", p=P))
                        nc.sync.dma_start(
                            wv_t[:], wv.rearrange("(t p) n -> p t n", p=P))
                        nc.sync.dma_start(cos_t[:], cosT[:])
                        nc.sync.dma_start(sin_t[:], sinST[:])

                    # wave q / wave k: transposed projection + RoPE
                    for wname, w_t, dst in (("q", wq_t, qT), ("k", wk_t, kT)):
                        ps = [pspool.tile([P, SB], F32, tag="psA",
                                          name=f"ps{wname}_{sb}_{h}")
                              for h in range(HPC)]
                        for dt in range(NDT):
                            for h in range(HPC):
                                nc.tensor.matmul(
                                    ps[h][:], w_t[:, dt, bass.ts(h, P)],
                                    xts[dt][:],
                                    start=(dt == 0), stop=(dt == NDT - 1))
                        for h in range(HPC):
                            p = ps[h]
                            tmp = tpool.tile([P, SB], F32, tag="ropetmp")
                            nc.vector.tensor_mul(
                                tmp[0:64, :], p[64:128, :], sin_t[0:64, ssl])
                            nc.vector.tensor_mul(
                                tmp[64:128, :], p[0:64, :], sin_t[64:128, ssl])
                            dst_ap = dst[:, h, ssl]
                            nc.vector.tensor_mul(dst_ap, p[:], cos_t[:, ssl])
                            nc.vector.tensor_add(dst_ap, dst_ap, tmp[:])
                            if with_qkv_bias:
                                bt = bqr_t if wname == "q" else bkr_t
                                nc.vector.tensor_add(dst_ap, dst_ap,
                                                     bt[:, h, ssl])

                    # wave v: natural projection, lhsT is a slice of xt
                    pv = [pspool.tile([P, DG], F32, tag="psA",
                                      name=f"psv_{sb}_{j}")
                          for j in range(4)]
                    for dt in range(NDT):
                        for j in range(4):
                            nc.tensor.matmul(
                                pv[j][:], xts[dt][:, bass.ts(j, P)],
                                wv_t[:, dt, :],
                                start=(dt == 0), stop=(dt == NDT - 1))
                    for j in range(4):
                        st = 4 * sb + j
                        if with_qkv_bias:
                            nc.vector.tensor_add(vN[:, st, :], pv[j][:],
                                                 bv_t[:])
                        else:
                            nc.vector.tensor_copy(vN[:, st, :], pv[j][:])

            # ---- phase C: causal attention per (head, i-block) ----
            # j-tiles processed in pairs; exp runs as one 1024-wide ACT op.
            oT_pool = top.enter_context(tc.tile_pool(name="oTp", bufs=1))
            oT = oT_pool.tile([P, HPC, S], MMDT, tag="oT")
            # prefetch the out-proj weights during attention
            wopool = top.enter_context(tc.tile_pool(name="wo", bufs=1))
            wo_t = wopool.tile([P, HPC, D], MMDT, tag="wo")
            for hh in range(HPC):
                nc.sync.dma_start(wo_t[:, hh, :], wo[bass.ts(hh, P), :])
            with ExitStack() as cctx:
                mpool = cctx.enter_context(tc.tile_pool(name="masks", bufs=1))
                mask_t = mpool.tile([P, 2, 2, SB], MMDT, tag="masks")
                nc.sync.dma_start(mask_t[:], masksT[:])
                ones_t = mpool.tile([P, P], MMDT, tag="ones")
                nc.sync.dma_start(ones_t[:], onesd[:])
                epool = cctx.enter_context(tc.tile_pool(name="et", bufs=4))
                rpool = cctx.enter_context(tc.tile_pool(name="recip", bufs=2))
                psS = cctx.enter_context(
                    tc.tile_pool(name="psS", bufs=2, space="PSUM"))
                psO = cctx.enter_context(
                    tc.tile_pool(name="psO", bufs=2, space="PSUM"))
                psD = cctx.enter_context(
                    tc.tile_pool(name="psD", bufs=2, space="PSUM"))

                for h in range(HPC):
                    for ib in range(NSB):
                        isl = bass.ts(ib, SB)
                        npair = 2 * (ib + 1)
                        po = psO.tile([P, SB], F32, tag="psO")
                        pd = psD.tile([P, SB], F32, tag="psD")
                        for pt in range(npair):
                            pss = psS.tile([P, 2, SB], F32, tag="psS")
                            for t in range(2):
                                nc.tensor.matmul(
                                    pss[:, t, :],
                                    kT[:, h, bass.ts(2 * pt + t, P)],
                                    qT[:, h, isl], start=True, stop=True)
                            et = epool.tile([P, 2, SB], MMDT, tag="et")
                            nc.scalar.activation(
                                et[:], pss[:],
                                mybir.ActivationFunctionType.Exp, scale=SCALE)
                            o = pt - 2 * ib
                            if o >= 0:
                                nc.vector.tensor_mul(et[:], et[:],
                                                     mask_t[:, o, :, :])
                            first = (pt == 0)
                            last = (pt == npair - 1)
                            for t in range(2):
                                nc.tensor.matmul(
                                    pd[:], ones_t[:], et[:, t, :],
                                    start=(first and t == 0),
                                    stop=(last and t == 1))
                                nc.tensor.matmul(
                                    po[:], vN[:, 2 * pt + t, bass.ts(h, P)],
                                    et[:, t, :],
                                    start=(first and t == 0),
                                    stop=(last and t == 1))
                        # 1/den on one row only, then matmul-broadcast (K=1)
                        # to all 128 partitions: lhsT=ones[0:1,:], rhs=rec1
                        # 1/den on one row, then GpSimd partition-broadcast
                        # (PE is in-order; a broadcast matmul here would
                        # bubble the next iteration's scores)
                        rec1 = rpool.tile([1, SB], F32, tag="recip")
                        nc.vector.reciprocal(rec1[:], pd[0:1, :])
                        recs = rpool.tile([P, SB], F32, tag="recs")
                        nc.gpsimd.partition_broadcast(recs[:], rec1[:])
                        nc.vector.tensor_mul(oT[:, h, isl], po[:], recs[:])

            # ---- phase D: partial output projection ----
            with ExitStack() as dctx:
                opool = dctx.enter_context(tc.tile_pool(name="outsb", bufs=4))
                psE = dctx.enter_context(
                    tc.tile_pool(name="psE", bufs=2, space="PSUM"))
                for st in range(NST):
                    for eb in range(NSB):
                        pe = psE.tile([P, SB], F32, tag="psE")
                        for hh in range(HPC):
                            nc.tensor.matmul(
                                pe[:], oT[:, hh, bass.ts(st, P)],
                                wo_t[:, hh, bass.ts(eb, SB)],
                                start=(hh == 0), stop=(hh == HPC - 1))
                        ob = opool.tile([P, SB], F32, tag="outsb")
                        nc.vector.tensor_copy(ob[:], pe[:])
                        nc.sync.dma_start(
                            out[bass.ts(st, P), bass.ts(eb, SB)], ob[:])

    nc.compile()
    return nc


def _rot_cols(w):
    """rotate_half applied to the last axis (head-dim columns) of w."""
    r = np.empty_like(w)
    r[..., : HD // 2] = -w[..., HD // 2:]
    r[..., HD // 2:] = w[..., : HD // 2]
    return r


def _host_inputs(x, cos, sin, qkv_w, qkv_b, with_qkv_bias):
    """Build the 8 per-core input maps."""
    # signed sin, transposed: sinS[d] = -sin[d] for d<64 else +sin[d]
    sinS = sin.copy()
    sinS[:, : HD // 2] *= -1.0
    cosT = np.ascontiguousarray(cos.T)
    sinST = np.ascontiguousarray(sinS.T)
    masks = np.zeros((P, 2, 2, SB), dtype=MMNP)
    jj = np.arange(P)[:, None]
    ii = np.arange(SB)[None, :]
    for o in (0, 1):
        for t in (0, 1):
            masks[:, o, t, :] = (((2 * o + t) * P + jj) <= ii).astype(MMNP)
    ones = np.ones((P, P), dtype=MMNP)

    xTb = [np.ascontiguousarray(x[b].T).astype(MMNP) for b in range(B)]
    qkv_w16 = qkv_w.astype(MMNP)
    in_maps = []
    for c in range(NCORES):
        b, g = divmod(c, 4)
        cols = slice(g * DG, (g + 1) * DG)
        im = {
            "xT": xTb[b],
            "wq": np.ascontiguousarray(qkv_w16[:, cols]),
            "wk": np.ascontiguousarray(qkv_w16[:, D:][:, cols]),
            "wv": np.ascontiguousarray(qkv_w16[:, 2 * D:][:, cols]),
            "wo": None,  # filled by caller (needs out_w)
            "cosT": cosT,
            "sinST": sinST,
            "masksT": masks,
            "ones": ones,
        }
        if with_qkv_bias:
            bq = qkv_b[cols]
            bk = qkv_b[D:][cols]
            bv = qkv_b[2 * D:][cols]
            # roped bias, transposed per head: [HD, HPC, S]
            def rope_bias(bvec):
                r = np.empty((P, HPC, S), dtype=np.float32)
                for h in range(HPC):
                    bh = bvec[h * HD:(h + 1) * HD]  # [HD]
                    rb = _rot_cols(bh[None, :])[0]
                    # b*cos + rot(b)*sin, as [HD, S]
                    r[:, h, :] = (bh[None, :] * cos + rb[None, :] * sin).T
                return r
            im["bqrope"] = rope_bias(bq)
            im["bkrope"] = rope_bias(bk)
            im["bv128"] = np.tile(bv[None, :], (P, 1)).astype(np.float32)
        in_maps.append(im)
    return in_maps


_CACHED = {}


def _get_program(with_qkv_bias):
    if with_qkv_bias not in _CACHED:
        _CACHED[with_qkv_bias] = _build_program(with_qkv_bias)
    return _CACHED[with_qkv_bias]


def run_on_cores(in_maps, profile_dir=None):
    """Execute the prebuilt program on 8 cores; optionally capture NTFF."""
    from concourse import bass2jax
    with_qkv_bias = "bqrope" in in_maps[0]
    nc = _get_program(with_qkv_bias)
    if profile_dir is not None:
        from trn_agent_boot.trn_boot import _ntff_profile_via_ctypes
        hook = _ntff_profile_via_ctypes("/opt/axon/libaxon_pjrt.so")
        with hook(profile_dir, [0]):
            results = bass2jax.run_bass_via_pjrt(nc, in_maps, n_cores=NCORES)
    else:
        results = bass2jax.run_bass_via_pjrt(nc, in_maps, n_cores=NCORES)
    return results


def kernel(x, cos, sin, qkv_w, qkv_b, out_w, out_b, _profile_dir=None):
    x = np.asarray(x, dtype=np.float32)
    cos = np.asarray(cos, dtype=np.float32)
    sin = np.asarray(sin, dtype=np.float32)
    qkv_w = np.asarray(qkv_w, dtype=np.float32)
    qkv_b = np.asarray(qkv_b, dtype=np.float32)
    out_w = np.asarray(out_w, dtype=np.float32)
    out_b = np.asarray(out_b, dtype=np.float32)

    with_qkv_bias = bool(np.any(qkv_b != 0))
    in_maps = _host_inputs(x, cos, sin, qkv_w, qkv_b, with_qkv_bias)
    for c in range(NCORES):
        g = c % 4
        in_maps[c]["wo"] = np.ascontiguousarray(
            out_w[g * DG:(g + 1) * DG, :]).astype(MMNP)

    results = run_on_cores(in_maps, profile_dir=_profile_dir)

    final = np.zeros((B, S, D), dtype=np.float32)
    for c in range(NCORES):
        b = c // 4
        final[b] += results[c]["out"]
    final += out_b[None, None, :]
    return final


# revision 25
# speedup vs baseline: 1.1908x; 1.0782x over previous
"""Causal self-attention with RoPE, tensor-parallel over (batch, head-group)
across 8 NeuronCores.

Sharding: core c = 4*b + g handles batch b (of 2) and head group g (of 4),
i.e. heads 4g..4g+3.  Each core computes q/k projections in transposed
layout [head_dim, seq] (weights become matmul lhsT naturally), v in natural
layout [seq, head_dim], applies RoPE, runs causal attention without
max-subtraction (scores are O(3), exp is safe in fp32), and emits a partial
output projection.  The host sums the 4 per-head-group partials per batch.

All matmul operands are fp16 (full PE rate, f32 PSUM accumulation); the
non-matmul math (RoPE, exp, reciprocal) stays f32.
"""

import sys
from contextlib import ExitStack

sys.path.insert(0, "/opt/trn_rl_repo")

import numpy as np

import concourse.bass as bass
import concourse.tile as tile
from concourse import bacc, mybir

B, S, D, H, HD = 2, 2048, 2048, 16, 128
NCORES = 8
HPC = H // 4  # heads per core = 4
DG = HPC * HD  # 512 cols per head group
P = 128
SB = 512  # s-block (matmul free dim)
NSB = S // SB  # 4
NDT = D // P  # 16 contraction tiles of the model dim
NST = S // P  # 16 seq tiles
F32 = mybir.dt.float32
F32R = mybir.dt.float32r
MMDT = mybir.dt.float16
MMNP = np.float16
SCALE = 1.0 / float(np.sqrt(HD))


def _build_program(with_qkv_bias: bool):
    nc = bacc.Bacc("TRN2", target_bir_lowering=False, debug=False,
                   num_devices=NCORES)
    xT = nc.dram_tensor("xT", [D, S], MMDT, kind="ExternalInput").ap()
    wq = nc.dram_tensor("wq", [D, DG], MMDT, kind="ExternalInput").ap()
    wk = nc.dram_tensor("wk", [D, DG], MMDT, kind="ExternalInput").ap()
    wv = nc.dram_tensor("wv", [D, DG], MMDT, kind="ExternalInput").ap()
    wo = nc.dram_tensor("wo", [DG, D], MMDT, kind="ExternalInput").ap()
    cosT = nc.dram_tensor("cosT", [P, S], F32, kind="ExternalInput").ap()
    sinST = nc.dram_tensor("sinST", [P, S], F32, kind="ExternalInput").ap()
    masksT = nc.dram_tensor("masksT", [P, 2, 2, SB], MMDT,
                            kind="ExternalInput").ap()
    onesd = nc.dram_tensor("ones", [P, P], MMDT, kind="ExternalInput").ap()
    if with_qkv_bias:
        bqr = nc.dram_tensor("bqrope", [P, HPC, S], F32, kind="ExternalInput").ap()
        bkr = nc.dram_tensor("bkrope", [P, HPC, S], F32, kind="ExternalInput").ap()
        bv128 = nc.dram_tensor("bv128", [P, DG], F32, kind="ExternalInput").ap()
    out = nc.dram_tensor("out", [S, D], F32, kind="ExternalOutput").ap()

    with tile.TileContext(nc) as tc:
        with ExitStack() as top:
            # ---- persistent tiles ----
            qkT_pool = top.enter_context(tc.tile_pool(name="qkT", bufs=1))
            qT = qkT_pool.tile([P, HPC, S], MMDT, tag="qT")
            kT = qkT_pool.tile([P, HPC, S], MMDT, tag="kT")
            v_pool = top.enter_context(tc.tile_pool(name="vp", bufs=1))
            vN = v_pool.tile([P, NST, DG], MMDT, tag="vN")

            # ---- phase A: q/k (transposed) + v (natural) projections ----
            # Three 4-bank PE waves per s-block (q, k, v); with 8 PSUM banks
            # two waves are in flight so RoPE/copy eviction of wave i
            # overlaps wave i+1's matmuls and the PE never idles.
            with ExitStack() as actx:
                wpool = actx.enter_context(tc.tile_pool(name="wqkv", bufs=1))
                wq_t = wpool.tile([P, NDT, DG], MMDT, tag="wq")
                wk_t = wpool.tile([P, NDT, DG], MMDT, tag="wk")
                wv_t = wpool.tile([P, NDT, DG], MMDT, tag="wv")
                # wq first; wk/wv/cos/sin issued after sb0's x tiles so the
                # first q matmuls aren't queued behind 7MB of other DMA
                nc.sync.dma_start(wq_t[:], wq.rearrange("(t p) n -> p t n", p=P))
                cpool = actx.enter_context(tc.tile_pool(name="cs", bufs=1))
                cos_t = cpool.tile([P, S], F32, tag="cos")
                sin_t = cpool.tile([P, S], F32, tag="sin")
                if with_qkv_bias:
                    bpool = actx.enter_context(tc.tile_pool(name="bqk", bufs=1))
                    bqr_t = bpool.tile([P, HPC, S], F32, tag="bqr")
                    bkr_t = bpool.tile([P, HPC, S], F32, tag="bkr")
                    bv_t = bpool.tile([P, DG], F32, tag="bv")
                    nc.sync.dma_start(bqr_t[:], bqr[:])
                    nc.sync.dma_start(bkr_t[:], bkr[:])
                    nc.sync.dma_start(bv_t[:], bv128[:])
                xpool = actx.enter_context(tc.tile_pool(name="xs", bufs=32))
                tpool = actx.enter_context(tc.tile_pool(name="ropetmp", bufs=4))
                pspool = actx.enter_context(
                    tc.tile_pool(name="psA", bufs=8, space="PSUM"))

                for sb in range(NSB):
                    ssl = bass.ts(sb, SB)
                    xts = []
                    for dt in range(NDT):
                        xt = xpool.tile([P, SB], MMDT, tag="xs",
                                        name=f"x_{sb}_{dt}")
                        nc.sync.dma_start(xt[:], xT[bass.ts(dt, P), ssl])
                        xts.append(xt)
                    if sb == 0:
                        nc.sync.dma_start(
                            wk_t[:], wk.rearrange("(t p) n -> p t n", p=P))
                        nc.sync.dma_start(
                            wv_t[:], wv.rearrange("(t p) n -> p t n", p=P))
                        nc.sync.dma_start(cos_t[:], cosT[:])
                        nc.sync.dma_start(sin_t[:], sinST[:])

                    # wave q / wave k: transposed projection + RoPE
                    for wname, w_t, dst in (("q", wq_t, qT), ("k", wk_t, kT)):
                        ps = [pspool.tile([P, SB], F32, tag="psA",
                                          name=f"ps{wname}_{sb}_{h}")
                              for h in range(HPC)]
                        for dt in range(NDT):
                            for h in range(HPC):
                                nc.tensor.matmul(
                                    ps[h][:], w_t[:, dt, bass.ts(h, P)],
                                    xts[dt][:],
                                    start=(dt == 0), stop=(dt == NDT - 1))
                        for h in range(HPC):
                            p = ps[h]
                            tmp = tpool.tile([P, SB], F32, tag="ropetmp")
                            nc.vector.tensor_mul(
                                tmp[0:64, :], p[64:128, :], sin_t[0:64, ssl])
                            nc.vector.tensor_mul(
                                tmp[64:128, :], p[0:64, :], sin_t[64:128, ssl])
                            dst_ap = dst[:, h, ssl]
                            nc.vector.tensor_mul(dst_ap, p[:], cos_t[:, ssl])
                            nc.vector.tensor_add(dst_ap, dst_ap, tmp[:])
                            if with_qkv_bias:
                                bt = bqr_t if wname == "q" else bkr_t
                                nc.vector.tensor_add(dst_ap, dst_ap,
                                                     bt[:, h, ssl])

                    # wave v: natural projection, lhsT is a slice of xt
                    pv = [pspool.tile([P, DG], F32, tag="psA",
                                      name=f"psv_{sb}_{j}")
                          for j in range(4)]
                    for dt in range(NDT):
                        for j in range(4):
                            nc.tensor.matmul(
                                pv[j][:], xts[dt][:, bass.ts(j, P)],
                                wv_t[:, dt, :],
                                start=(dt == 0), stop=(dt == NDT - 1))
                    for j in range(4):
                        st = 4 * sb + j
                        if with_qkv_bias:
                            nc.vector.tensor_add(vN[:, st, :], pv[j][:],
                                                 bv_t[:])
                        else:
                            nc.vector.tensor_copy(vN[:, st, :], pv[j][:])

            # ---- phase C: causal attention per (head, i-block) ----
            # j-tiles processed in pairs; exp runs as one 1024-wide ACT op.
            oT_pool = top.enter_context(tc.tile_pool(name="oTp", bufs=1))
            oT = oT_pool.tile([P, HPC, S], MMDT, tag="oT")
            # prefetch the out-proj weights during attention
            wopool = top.enter_context(tc.tile_pool(name="wo", bufs=1))
            wo_t = wopool.tile([P, HPC, D], MMDT, tag="wo")
            for hh in range(HPC):
                nc.sync.dma_start(wo_t[:, hh, :], wo[bass.ts(hh, P), :])
            with ExitStack() as cctx:
                mpool = cctx.enter_context(tc.tile_pool(name="masks", bufs=1))
                mask_t = mpool.tile([P, 2, 2, SB], MMDT, tag="masks")
                nc.sync.dma_start(mask_t[:], masksT[:])
                ones_t = mpool.tile([P, P], MMDT, tag="ones")
                nc.sync.dma_start(ones_t[:], onesd[:])
                epool = cctx.enter_context(tc.tile_pool(name="et", bufs=4))
                rpool = cctx.enter_context(tc.tile_pool(name="recip", bufs=2))
                psS = cctx.enter_context(
                    tc.tile_pool(name="psS", bufs=2, space="PSUM"))
                psO = cctx.enter_context(
                    tc.tile_pool(name="psO", bufs=2, space="PSUM"))
                psD = cctx.enter_context(
                    tc.tile_pool(name="psD", bufs=2, space="PSUM"))

                for h in range(HPC):
                    for ib in range(NSB):
                        isl = bass.ts(ib, SB)
                        npair = 2 * (ib + 1)
                        po = psO.tile([P, SB], F32, tag="psO")
                        pd = psD.tile([P, SB], F32, tag="psD")

                        # software pipeline: issue scores(pt+1) before
                        # den/av(pt) so the PE (in-order) overlaps the exp
                        # latency of pair pt with real work
                        def scores_exp(pt):
                            pss = psS.tile([P, 2, SB], F32, tag="psS",
                                           name=f"pss_{h}_{ib}_{pt}")
                            for t in range(2):
                                nc.tensor.matmul(
                                    pss[:, t, :],
                                    kT[:, h, bass.ts(2 * pt + t, P)],
                                    qT[:, h, isl], start=True, stop=True)
                            et = epool.tile([P, 2, SB], MMDT, tag="et",
                                            name=f"et_{h}_{ib}_{pt}")
                            nc.scalar.activation(
                                et[:], pss[:],
                                mybir.ActivationFunctionType.Exp, scale=SCALE)
                            o = pt - 2 * ib
                            if o >= 0:
                                nc.vector.tensor_mul(et[:], et[:],
                                                     mask_t[:, o, :, :])
                            return et

                        ets = {0: scores_exp(0)}
                        for pt in range(npair):
                            if pt + 1 < npair:
                                ets[pt + 1] = scores_exp(pt + 1)
                            et = ets.pop(pt)
                            first = (pt == 0)
                            last = (pt == npair - 1)
                            for t in range(2):
                                nc.tensor.matmul(
                                    pd[:], ones_t[:], et[:, t, :],
                                    start=(first and t == 0),
                                    stop=(last and t == 1))
                                nc.tensor.matmul(
                                    po[:], vN[:, 2 * pt + t, bass.ts(h, P)],
                                    et[:, t, :],
                                    start=(first and t == 0),
                                    stop=(last and t == 1))
                        # 1/den on one row only, then matmul-broadcast (K=1)
                        # to all 128 partitions: lhsT=ones[0:1,:], rhs=rec1
                        # 1/den on one row, then GpSimd partition-broadcast
                        # (PE is in-order; a broadcast matmul here would
                        # bubble the next iteration's scores)
                        rec1 = rpool.tile([1, SB], F32, tag="recip")
                        nc.vector.reciprocal(rec1[:], pd[0:1, :])
                        recs = rpool.tile([P, SB], F32, tag="recs")
                        nc.gpsimd.partition_broadcast(recs[:], rec1[:])
                        nc.vector.tensor_mul(oT[:, h, isl], po[:], recs[:])

            # ---- phase D: partial output projection ----
            with ExitStack() as dctx:
                opool = dctx.enter_context(tc.tile_pool(name="outsb", bufs=6))
                psE = dctx.enter_context(
                    tc.tile_pool(name="psE", bufs=3, space="PSUM"))
                for st in range(NST):
                    for eb in range(NSB):
                        pe = psE.tile([P, SB], F32, tag="psE")
                        for hh in range(HPC):
                            nc.tensor.matmul(
                                pe[:], oT[:, hh, bass.ts(st, P)],
                                wo_t[:, hh, bass.ts(eb, SB)],
                                start=(hh == 0), stop=(hh == HPC - 1))
                        ob = opool.tile([P, SB], F32, tag="outsb")
                        nc.vector.tensor_copy(ob[:], pe[:])
                        nc.sync.dma_start(
                            out[bass.ts(st, P), bass.ts(eb, SB)], ob[:])

    nc.compile()
    return nc


def _rot_cols(w):
    """rotate_half applied to the last axis (head-dim columns) of w."""
    r = np.empty_like(w)
    r[..., : HD // 2] = -w[..., HD // 2:]
    r[..., HD // 2:] = w[..., : HD // 2]
    return r


def _host_inputs(x, cos, sin, qkv_w, qkv_b, with_qkv_bias):
    """Build the 8 per-core input maps."""
    # signed sin, transposed: sinS[d] = -sin[d] for d<64 else +sin[d]
    sinS = sin.copy()
    sinS[:, : HD // 2] *= -1.0
    cosT = np.ascontiguousarray(cos.T)
    sinST = np.ascontiguousarray(sinS.T)
    masks = np.zeros((P, 2, 2, SB), dtype=MMNP)
    jj = np.arange(P)[:, None]
    ii = np.arange(SB)[None, :]
    for o in (0, 1):
        for t in (0, 1):
            masks[:, o, t, :] = (((2 * o + t) * P + jj) <= ii).astype(MMNP)
    ones = np.ones((P, P), dtype=MMNP)

    xTb = [np.ascontiguousarray(x[b].T).astype(MMNP) for b in range(B)]
    qkv_w16 = qkv_w.astype(MMNP)
    in_maps = []
    for c in range(NCORES):
        b, g = divmod(c, 4)
        cols = slice(g * DG, (g + 1) * DG)
        im = {
            "xT": xTb[b],
            "wq": np.ascontiguousarray(qkv_w16[:, cols]),
            "wk": np.ascontiguousarray(qkv_w16[:, D:][:, cols]),
            "wv": np.ascontiguousarray(qkv_w16[:, 2 * D:][:, cols]),
            "wo": None,  # filled by caller (needs out_w)
            "cosT": cosT,
            "sinST": sinST,
            "masksT": masks,
            "ones": ones,
        }
        if with_qkv_bias:
            bq = qkv_b[cols]
            bk = qkv_b[D:][cols]
            bv = qkv_b[2 * D:][cols]
            # roped bias, transposed per head: [HD, HPC, S]
            def rope_bias(bvec):
                r = np.empty((P, HPC, S), dtype=np.float32)
                for h in range(HPC):
                    bh = bvec[h * HD:(h + 1) * HD]  # [HD]
                    rb = _rot_cols(bh[None, :])[0]
                    # b*cos + rot(b)*sin, as [HD, S]
                    r[:, h, :] = (bh[None, :] * cos + rb[None, :] * sin).T
                return r
            im["bqrope"] = rope_bias(bq)
            im["bkrope"] = rope_bias(bk)
            im["bv128"] = np.tile(bv[None, :], (P, 1)).astype(np.float32)
        in_maps.append(im)
    return in_maps


_CACHED = {}


def _get_program(with_qkv_bias):
    if with_qkv_bias not in _CACHED:
        _CACHED[with_qkv_bias] = _build_program(with_qkv_bias)
    return _CACHED[with_qkv_bias]


def run_on_cores(in_maps, profile_dir=None):
    """Execute the prebuilt program on 8 cores; optionally capture NTFF."""
    from concourse import bass2jax
    with_qkv_bias = "bqrope" in in_maps[0]
    nc = _get_program(with_qkv_bias)
    if profile_dir is not None:
        from trn_agent_boot.trn_boot import _ntff_profile_via_ctypes
        hook = _ntff_profile_via_ctypes("/opt/axon/libaxon_pjrt.so")
        with hook(profile_dir, [0]):
            results = bass2jax.run_bass_via_pjrt(nc, in_maps, n_cores=NCORES)
    else:
        results = bass2jax.run_bass_via_pjrt(nc, in_maps, n_cores=NCORES)
    return results


def kernel(x, cos, sin, qkv_w, qkv_b, out_w, out_b, _profile_dir=None):
    x = np.asarray(x, dtype=np.float32)
    cos = np.asarray(cos, dtype=np.float32)
    sin = np.asarray(sin, dtype=np.float32)
    qkv_w = np.asarray(qkv_w, dtype=np.float32)
    qkv_b = np.asarray(qkv_b, dtype=np.float32)
    out_w = np.asarray(out_w, dtype=np.float32)
    out_b = np.asarray(out_b, dtype=np.float32)

    with_qkv_bias = bool(np.any(qkv_b != 0))
    in_maps = _host_inputs(x, cos, sin, qkv_w, qkv_b, with_qkv_bias)
    for c in range(NCORES):
        g = c % 4
        in_maps[c]["wo"] = np.ascontiguousarray(
            out_w[g * DG:(g + 1) * DG, :]).astype(MMNP)

    results = run_on_cores(in_maps, profile_dir=_profile_dir)

    final = np.zeros((B, S, D), dtype=np.float32)
    for c in range(NCORES):
        b = c // 4
        final[b] += results[c]["out"]
    final += out_b[None, None, :]
    return final


# revision 27
# speedup vs baseline: 1.2985x; 1.0905x over previous
"""Causal self-attention with RoPE, tensor-parallel over (batch, head-group)
across 8 NeuronCores.

Sharding: core c = 4*b + g handles batch b (of 2) and head group g (of 4),
i.e. heads 4g..4g+3.  Each core computes q/k projections in transposed
layout [head_dim, seq] (weights become matmul lhsT naturally), v in natural
layout [seq, head_dim], applies RoPE, runs causal attention without
max-subtraction (scores are O(3), exp is safe in fp32), and emits a partial
output projection.  The host sums the 4 per-head-group partials per batch.

All matmul operands are fp16 (full PE rate, f32 PSUM accumulation); the
non-matmul math (RoPE, exp, reciprocal) stays f32.
"""

import sys
from contextlib import ExitStack

sys.path.insert(0, "/opt/trn_rl_repo")

import numpy as np

import concourse.bass as bass
import concourse.tile as tile
from concourse import bacc, mybir

B, S, D, H, HD = 2, 2048, 2048, 16, 128
NCORES = 8
HPC = H // 4  # heads per core = 4
DG = HPC * HD  # 512 cols per head group
P = 128
SB = 512  # s-block (matmul free dim)
NSB = S // SB  # 4
NDT = D // P  # 16 contraction tiles of the model dim
NST = S // P  # 16 seq tiles
F32 = mybir.dt.float32
F32R = mybir.dt.float32r
MMDT = mybir.dt.float16
MMNP = np.float16
SCALE = 1.0 / float(np.sqrt(HD))


def _build_program(with_qkv_bias: bool):
    nc = bacc.Bacc("TRN2", target_bir_lowering=False, debug=False,
                   num_devices=NCORES)
    xT = nc.dram_tensor("xT", [D, S], MMDT, kind="ExternalInput").ap()
    wq = nc.dram_tensor("wq", [D, DG], MMDT, kind="ExternalInput").ap()
    wk = nc.dram_tensor("wk", [D, DG], MMDT, kind="ExternalInput").ap()
    wv = nc.dram_tensor("wv", [D, DG], MMDT, kind="ExternalInput").ap()
    wo = nc.dram_tensor("wo", [DG, D], MMDT, kind="ExternalInput").ap()
    cosT = nc.dram_tensor("cosT", [P, S], F32, kind="ExternalInput").ap()
    sinST = nc.dram_tensor("sinST", [P, S], F32, kind="ExternalInput").ap()
    masksT = nc.dram_tensor("masksT", [P, 2, 2, SB], MMDT,
                            kind="ExternalInput").ap()
    onesd = nc.dram_tensor("ones", [P, P], MMDT, kind="ExternalInput").ap()
    if with_qkv_bias:
        bqr = nc.dram_tensor("bqrope", [P, HPC, S], F32, kind="ExternalInput").ap()
        bkr = nc.dram_tensor("bkrope", [P, HPC, S], F32, kind="ExternalInput").ap()
        bv128 = nc.dram_tensor("bv128", [P, DG], F32, kind="ExternalInput").ap()
    out = nc.dram_tensor("out", [S, D], F32, kind="ExternalOutput").ap()

    with tile.TileContext(nc) as tc:
        with ExitStack() as top:
            # ---- persistent tiles ----
            qkT_pool = top.enter_context(tc.tile_pool(name="qkT", bufs=1))
            qT = qkT_pool.tile([P, HPC, S], MMDT, tag="qT")
            kT = qkT_pool.tile([P, HPC, S], MMDT, tag="kT")
            v_pool = top.enter_context(tc.tile_pool(name="vp", bufs=1))
            vN = v_pool.tile([P, NST, DG], MMDT, tag="vN")

            # ---- phase A: q/k (transposed) + v (natural) projections ----
            # Three 4-bank PE waves per s-block (q, k, v); with 8 PSUM banks
            # two waves are in flight so RoPE/copy eviction of wave i
            # overlaps wave i+1's matmuls and the PE never idles.
            with ExitStack() as actx:
                wpool = actx.enter_context(tc.tile_pool(name="wqkv", bufs=1))
                wq_t = wpool.tile([P, NDT, DG], MMDT, tag="wq")
                wk_t = wpool.tile([P, NDT, DG], MMDT, tag="wk")
                wv_t = wpool.tile([P, NDT, DG], MMDT, tag="wv")
                # wq first; wk/wv/cos/sin issued after sb0's x tiles so the
                # first q matmuls aren't queued behind 7MB of other DMA
                nc.sync.dma_start(wq_t[:], wq.rearrange("(t p) n -> p t n", p=P))
                cpool = actx.enter_context(tc.tile_pool(name="cs", bufs=1))
                cos_t = cpool.tile([P, S], F32, tag="cos")
                sin_t = cpool.tile([P, S], F32, tag="sin")
                if with_qkv_bias:
                    bpool = actx.enter_context(tc.tile_pool(name="bqk", bufs=1))
                    bqr_t = bpool.tile([P, HPC, S], F32, tag="bqr")
                    bkr_t = bpool.tile([P, HPC, S], F32, tag="bkr")
                    bv_t = bpool.tile([P, DG], F32, tag="bv")
                    nc.sync.dma_start(bqr_t[:], bqr[:])
                    nc.sync.dma_start(bkr_t[:], bkr[:])
                    nc.sync.dma_start(bv_t[:], bv128[:])
                xpool = actx.enter_context(tc.tile_pool(name="xs", bufs=32))
                tpool = actx.enter_context(tc.tile_pool(name="ropetmp", bufs=4))
                pspool = actx.enter_context(
                    tc.tile_pool(name="psA", bufs=8, space="PSUM"))

                for sb in range(NSB):
                    ssl = bass.ts(sb, SB)
                    xts = []
                    for dt in range(NDT):
                        xt = xpool.tile([P, SB], MMDT, tag="xs",
                                        name=f"x_{sb}_{dt}")
                        nc.sync.dma_start(xt[:], xT[bass.ts(dt, P), ssl])
                        xts.append(xt)
                    if sb == 0:
                        nc.sync.dma_start(
                            wk_t[:], wk.rearrange("(t p) n -> p t n", p=P))
                        nc.sync.dma_start(
                            wv_t[:], wv.rearrange("(t p) n -> p t n", p=P))
                        nc.sync.dma_start(cos_t[:], cosT[:])
                        nc.sync.dma_start(sin_t[:], sinST[:])

                    # wave q / wave k: transposed projection + RoPE
                    for wname, w_t, dst in (("q", wq_t, qT), ("k", wk_t, kT)):
                        ps = [pspool.tile([P, SB], F32, tag="psA",
                                          name=f"ps{wname}_{sb}_{h}")
                              for h in range(HPC)]
                        for dt in range(NDT):
                            for h in range(HPC):
                                nc.tensor.matmul(
                                    ps[h][:], w_t[:, dt, bass.ts(h, P)],
                                    xts[dt][:],
                                    start=(dt == 0), stop=(dt == NDT - 1))
                        for h in range(HPC):
                            p = ps[h]
                            tmp = tpool.tile([P, SB], F32, tag="ropetmp")
                            nc.vector.tensor_mul(
                                tmp[0:64, :], p[64:128, :], sin_t[0:64, ssl])
                            nc.vector.tensor_mul(
                                tmp[64:128, :], p[0:64, :], sin_t[64:128, ssl])
                            dst_ap = dst[:, h, ssl]
                            nc.vector.tensor_mul(dst_ap, p[:], cos_t[:, ssl])
                            nc.vector.tensor_add(dst_ap, dst_ap, tmp[:])
                            if with_qkv_bias:
                                bt = bqr_t if wname == "q" else bkr_t
                                nc.vector.tensor_add(dst_ap, dst_ap,
                                                     bt[:, h, ssl])

                    # wave v: natural projection, lhsT is a slice of xt
                    pv = [pspool.tile([P, DG], F32, tag="psA",
                                      name=f"psv_{sb}_{j}")
                          for j in range(4)]
                    for dt in range(NDT):
                        for j in range(4):
                            nc.tensor.matmul(
                                pv[j][:], xts[dt][:, bass.ts(j, P)],
                                wv_t[:, dt, :],
                                start=(dt == 0), stop=(dt == NDT - 1))
                    for j in range(4):
                        st = 4 * sb + j
                        if with_qkv_bias:
                            nc.vector.tensor_add(vN[:, st, :], pv[j][:],
                                                 bv_t[:])
                        else:
                            nc.vector.tensor_copy(vN[:, st, :], pv[j][:])

            # ---- phase C: causal attention per (head, i-block) ----
            # j-tiles processed in pairs; exp runs as one 1024-wide ACT op.
            oT_pool = top.enter_context(tc.tile_pool(name="oTp", bufs=1))
            oT = oT_pool.tile([P, HPC, S], MMDT, tag="oT")
            # prefetch the out-proj weights during attention
            wopool = top.enter_context(tc.tile_pool(name="wo", bufs=1))
            wo_t = wopool.tile([P, HPC, D], MMDT, tag="wo")
            for hh in range(HPC):
                nc.sync.dma_start(wo_t[:, hh, :], wo[bass.ts(hh, P), :])
            with ExitStack() as cctx:
                mpool = cctx.enter_context(tc.tile_pool(name="masks", bufs=1))
                mask_t = mpool.tile([P, 2, 2, SB], MMDT, tag="masks")
                nc.sync.dma_start(mask_t[:], masksT[:])
                ones_t = mpool.tile([P, P], MMDT, tag="ones")
                nc.sync.dma_start(ones_t[:], onesd[:])
                epool = cctx.enter_context(tc.tile_pool(name="et", bufs=6))
                rpool = cctx.enter_context(tc.tile_pool(name="recip", bufs=2))
                psS = cctx.enter_context(
                    tc.tile_pool(name="psS", bufs=2, space="PSUM"))
                psO = cctx.enter_context(
                    tc.tile_pool(name="psO", bufs=2, space="PSUM"))
                psD = cctx.enter_context(
                    tc.tile_pool(name="psD", bufs=2, space="PSUM"))

                for h in range(HPC):
                    for ib in range(NSB):
                        isl = bass.ts(ib, SB)
                        npair = 2 * (ib + 1)
                        po = psO.tile([P, SB], F32, tag="psO")
                        pd = psD.tile([P, SB], F32, tag="psD")

                        # software pipeline: issue scores(pt+1) before
                        # den/av(pt) so the PE (in-order) overlaps the exp
                        # latency of pair pt with real work
                        def scores_exp(pt):
                            pss = psS.tile([P, 2, SB], F32, tag="psS",
                                           name=f"pss_{h}_{ib}_{pt}")
                            for t in range(2):
                                nc.tensor.matmul(
                                    pss[:, t, :],
                                    kT[:, h, bass.ts(2 * pt + t, P)],
                                    qT[:, h, isl], start=True, stop=True)
                            et = epool.tile([P, 2, SB], MMDT, tag="et",
                                            name=f"et_{h}_{ib}_{pt}")
                            nc.scalar.activation(
                                et[:], pss[:],
                                mybir.ActivationFunctionType.Exp, scale=SCALE)
                            o = pt - 2 * ib
                            if o >= 0:
                                nc.vector.tensor_mul(et[:], et[:],
                                                     mask_t[:, o, :, :])
                            return et

                        ets = {0: scores_exp(0)}
                        for pt in range(npair):
                            if pt + 1 < npair:
                                ets[pt + 1] = scores_exp(pt + 1)
                            et = ets.pop(pt)
                            first = (pt == 0)
                            last = (pt == npair - 1)
                            for t in range(2):
                                nc.tensor.matmul(
                                    pd[:], ones_t[:], et[:, t, :],
                                    start=(first and t == 0),
                                    stop=(last and t == 1))
                                nc.tensor.matmul(
                                    po[:], vN[:, 2 * pt + t, bass.ts(h, P)],
                                    et[:, t, :],
                                    start=(first and t == 0),
                                    stop=(last and t == 1))
                        # 1/den on one row only, then matmul-broadcast (K=1)
                        # to all 128 partitions: lhsT=ones[0:1,:], rhs=rec1
                        # 1/den on one row, then GpSimd partition-broadcast
                        # (PE is in-order; a broadcast matmul here would
                        # bubble the next iteration's scores)
                        rec1 = rpool.tile([1, SB], F32, tag="recip")
                        nc.vector.reciprocal_approx_fast(rec1[:], pd[0:1, :])
                        recs = rpool.tile([P, SB], F32, tag="recs")
                        nc.gpsimd.partition_broadcast(recs[:], rec1[:])
                        nc.vector.tensor_mul(oT[:, h, isl], po[:], recs[:])

            # ---- phase D: partial output projection ----
            with ExitStack() as dctx:
                opool = dctx.enter_context(tc.tile_pool(name="outsb", bufs=6))
                psE = dctx.enter_context(
                    tc.tile_pool(name="psE", bufs=3, space="PSUM"))
                for st in range(NST):
                    for eb in range(NSB):
                        pe = psE.tile([P, SB], F32, tag="psE")
                        for hh in range(HPC):
                            nc.tensor.matmul(
                                pe[:], oT[:, hh, bass.ts(st, P)],
                                wo_t[:, hh, bass.ts(eb, SB)],
                                start=(hh == 0), stop=(hh == HPC - 1))
                        ob = opool.tile([P, SB], F32, tag="outsb")
                        nc.vector.tensor_copy(ob[:], pe[:])
                        nc.sync.dma_start(
                            out[bass.ts(st, P), bass.ts(eb, SB)], ob[:])

    nc.compile()
    return nc


def _rot_cols(w):
    """rotate_half applied to the last axis (head-dim columns) of w."""
    r = np.empty_like(w)
    r[..., : HD // 2] = -w[..., HD // 2:]
    r[..., HD // 2:] = w[..., : HD // 2]
    return r


def _host_inputs(x, cos, sin, qkv_w, qkv_b, with_qkv_bias):
    """Build the 8 per-core input maps."""
    # signed sin, transposed: sinS[d] = -sin[d] for d<64 else +sin[d]
    sinS = sin.copy()
    sinS[:, : HD // 2] *= -1.0
    cosT = np.ascontiguousarray(cos.T)
    sinST = np.ascontiguousarray(sinS.T)
    masks = np.zeros((P, 2, 2, SB), dtype=MMNP)
    jj = np.arange(P)[:, None]
    ii = np.arange(SB)[None, :]
    for o in (0, 1):
        for t in (0, 1):
            masks[:, o, t, :] = (((2 * o + t) * P + jj) <= ii).astype(MMNP)
    ones = np.ones((P, P), dtype=MMNP)

    xTb = [np.ascontiguousarray(x[b].T).astype(MMNP) for b in range(B)]
    qkv_w16 = qkv_w.astype(MMNP)
    in_maps = []
    for c in range(NCORES):
        b, g = divmod(c, 4)
        cols = slice(g * DG, (g + 1) * DG)
        im = {
            "xT": xTb[b],
            "wq": np.ascontiguousarray(qkv_w16[:, cols]),
            "wk": np.ascontiguousarray(qkv_w16[:, D:][:, cols]),
            "wv": np.ascontiguousarray(qkv_w16[:, 2 * D:][:, cols]),
            "wo": None,  # filled by caller (needs out_w)
            "cosT": cosT,
            "sinST": sinST,
            "masksT": masks,
            "ones": ones,
        }
        if with_qkv_bias:
            bq = qkv_b[cols]
            bk = qkv_b[D:][cols]
            bv = qkv_b[2 * D:][cols]
            # roped bias, transposed per head: [HD, HPC, S]
            def rope_bias(bvec):
                r = np.empty((P, HPC, S), dtype=np.float32)
                for h in range(HPC):
                    bh = bvec[h * HD:(h + 1) * HD]  # [HD]
                    rb = _rot_cols(bh[None, :])[0]
                    # b*cos + rot(b)*sin, as [HD, S]
                    r[:, h, :] = (bh[None, :] * cos + rb[None, :] * sin).T
                return r
            im["bqrope"] = rope_bias(bq)
            im["bkrope"] = rope_bias(bk)
            im["bv128"] = np.tile(bv[None, :], (P, 1)).astype(np.float32)
        in_maps.append(im)
    return in_maps


_CACHED = {}


def _get_program(with_qkv_bias):
    if with_qkv_bias not in _CACHED:
        _CACHED[with_qkv_bias] = _build_program(with_qkv_bias)
    return _CACHED[with_qkv_bias]


def run_on_cores(in_maps, profile_dir=None):
    """Execute the prebuilt program on 8 cores; optionally capture NTFF."""
    from concourse import bass2jax
    with_qkv_bias = "bqrope" in in_maps[0]
    nc = _get_program(with_qkv_bias)
    if profile_dir is not None:
        from trn_agent_boot.trn_boot import _ntff_profile_via_ctypes
        hook = _ntff_profile_via_ctypes("/opt/axon/libaxon_pjrt.so")
        with hook(profile_dir, [0]):
            results = bass2jax.run_bass_via_pjrt(nc, in_maps, n_cores=NCORES)
    else:
        results = bass2jax.run_bass_via_pjrt(nc, in_maps, n_cores=NCORES)
    return results


def kernel(x, cos, sin, qkv_w, qkv_b, out_w, out_b, _profile_dir=None):
    x = np.asarray(x, dtype=np.float32)
    cos = np.asarray(cos, dtype=np.float32)
    sin = np.asarray(sin, dtype=np.float32)
    qkv_w = np.asarray(qkv_w, dtype=np.float32)
    qkv_b = np.asarray(qkv_b, dtype=np.float32)
    out_w = np.asarray(out_w, dtype=np.float32)
    out_b = np.asarray(out_b, dtype=np.float32)

    with_qkv_bias = bool(np.any(qkv_b != 0))
    in_maps = _host_inputs(x, cos, sin, qkv_w, qkv_b, with_qkv_bias)
    for c in range(NCORES):
        g = c % 4
        in_maps[c]["wo"] = np.ascontiguousarray(
            out_w[g * DG:(g + 1) * DG, :]).astype(MMNP)

    results = run_on_cores(in_maps, profile_dir=_profile_dir)

    final = np.zeros((B, S, D), dtype=np.float32)
    for c in range(NCORES):
        b = c // 4
        final[b] += results[c]["out"]
    final += out_b[None, None, :]
    return final
